# revision 89
# baseline (speedup 1.0000x reference)
"""EuclideanDeconf kernel for 8x TRN2 NeuronCores.

Computes out[b, c] = (2/D) * x @ W.T - ||x||^2/D - ||W||^2/D
for x [16384, 1024] f32, W [2048, 1024] f32 -> out [16384, 2048] f32.

Sharding: data-parallel over the batch dim. Each of the 8 cores gets 2048
rows of x and the full W. The host does layout-only work (transpose /
cast / shard / concat); all FLOPs (matmul, row/col norms, combine) run
on device.

I/O dtypes (ring total 16 MiB/core ~= 46us, under the PE's ~57us fp8
GEMM+extras floor):
  xT8  e4m3 x16-prescaled, b-256-blocked [8, p, k, 256] (matmul lhsT,
       straight from DMA to LDWEIGHTS -- no on-device casts)
  xT16 f16, b-256-blocked [8, p, k, 256] (x^2 path; ~1e-5 rel err)
  wT   e4m3 x16-prescaled, p-major [p, k, c]
  y    f16, host-upcast to f32 (~5e-4 max rel)
All host layouts are p-major with the transferred block contiguous per
partition, so every DMA is a 2D pattern (128 descriptors, 2-16KB each):
descriptor generation on the Sync engine stays ~0.7us per transfer
(3D patterns cost 3-4.5us each and serialized the old startup).

Measured vs the fp32 reference: max rel err ~1.5e-3, norm ~3.1e-4
(gate 2e-2). HW exec ~95.6-96.1us across runs (baseline 124us);
residual variance is HAM power-throttle phase, which duty-limits the
PE to ~75%: run-to-run the half-duty windows land on different
b-tiles.

Ring order: x16 block0 (b0/b1 x^2 data squares uncontended before the
GEMM starts: the square -> tree -> dot -> copy chain feeds pass1's
bias and pass1 is what frees PSUM), then W k-groups + xT8 blk0/blk1
(W arrival paces b0's k-steps), then the rest round-robin.

Engine schedule (per core):
  PE:   6 full-width warmup matmuls (these make HAM grant full duty by
        the time real work lands -- dropping to 2 or going narrow
        measurably hurts), then b-tiles 0..15 back-to-back: 16 DR
        matmuls + x2 dot (bf16 t1 so its LDWEIGHTS runs 1 cyc/row)
        each; after b7: 16 DR w2-reduce + 4 replicate matmuls. The
        last b-tile runs a fused per-half pass1->pass2->store tail.
  ACT:  per tile: x2c copy + pass1 (t = s*psum - x2[b], f16) as four
        single-bank [128,512] PSUM drains (pmain bufs=6: banks free one
        ~0.6us op at a time, so the next tile's matmuls find all four
        of theirs free; measured +2.1us vs two [128,1024] tiles); one
        W^2 square slice after pass1 on odd tiles b1..b7 (keeps ACT
        cadence ~= PE cadence).
  DVE:  x2 k-trees; W^2 squares k0..k3 interleaved with chunk-0 trees;
        pass2 (y = t - w2[c], all-f16, 2x 16-bit rate) inline from b8,
        deferred b0..b7 (epool=20 t-tiles of runway) until w2rep
        exists.
  GPSIMD: x^2 squares from f16 (fp8 squares measured 4-6us/slice on
        DVE/GPSIMD vs 2us on ACT -- never put wsq on gpsimd).
"""

import numpy as np
import ml_dtypes

# Problem constants (hardcoded; kernel.py must be self-contained).
B, D, C = 16384, 1024, 2048
NCORES = 8
BSH = B // NCORES  # 2048 rows of x per core
P = 128            # partitions
KT = D // P        # 8 contraction tiles
BCH = 512          # x16 chunk columns
NCH = BSH // BCH   # 4 chunks
BLK = 256          # xT8 block columns (2 b-tiles)
NBLK = BSH // BLK  # 8 blocks
JT = BCH // P      # 4 b-tiles per chunk
NJ = BSH // P      # 16 b-tiles

_CACHE = {}


def _build_nc():
    import concourse.tile as tile
    import concourse.mybir as mybir
    import concourse.bass as bass
    from concourse import bacc

    f32 = mybir.dt.float32
    f16 = mybir.dt.float16
    bf16 = mybir.dt.bfloat16
    fp8 = mybir.dt.float8e4
    PSUM = bass.MemorySpace.PSUM
    Identity = mybir.ActivationFunctionType.Identity
    Copy = mybir.ActivationFunctionType.Copy
    Square = mybir.ActivationFunctionType.Square
    MULT = mybir.AluOpType.mult
    ADD = mybir.AluOpType.add
    DR = mybir.MatmulPerfMode.DoubleRow

    # x and W both host-prescaled by 16 (keeps e4m3 out of subnormals);
    # the epilogue scales fold the 1/256 back out.
    cross_scale = 2.0 / D / 256.0
    w2_scale = 1.0 / D / 256.0

    nc = bacc.Bacc(
        "TRN2",
        target_bir_lowering=False,
        debug=False,
        enable_asserts=False,
    )
    xT8 = nc.dram_tensor("xT8", [NBLK * P, KT * BLK], fp8,
                         kind="ExternalInput").ap()
    xT16 = nc.dram_tensor("xT16", [NBLK * P, KT * BLK], f16,
                          kind="ExternalInput").ap()
    wT = nc.dram_tensor("wT", [P, KT * C], fp8, kind="ExternalInput").ap()
    y = nc.dram_tensor("y", [BSH, C], f16, kind="ExternalOutput").ap()

    xT8r = xT8.rearrange("(s p) (k b) -> s p k b", p=P, k=KT)
    xT16r = xT16.rearrange("(s p) (k b) -> s p k b", p=P, k=KT)
    wTr = wT.rearrange("p (k c) -> p k c", k=KT)

    with tile.TileContext(nc) as tc:
        with (
            tc.tile_pool(name="consts", bufs=1) as cpool,
            tc.tile_pool(name="wpool", bufs=1) as wpool,
            tc.tile_pool(name="xpool", bufs=1) as xpool,
            tc.tile_pool(name="x16pool", bufs=3) as x16pool,
            tc.tile_pool(name="xsqpool", bufs=3) as xsqpool,
            tc.tile_pool(name="epool", bufs=22) as epool,
            tc.tile_pool(name="ypool", bufs=8) as ypool,
            tc.tile_pool(name="spool", bufs=8) as spool,
            tc.tile_pool(name="pmain", bufs=6, space=PSUM) as pmain,
            tc.tile_pool(name="psmall", bufs=1, space=PSUM) as psmall,
        ):
            negones_f = cpool.tile([P, 1], f32)
            nc.gpsimd.memset(negones_f[:], -1.0)
            negones_b = cpool.tile([P, 1], bf16)
            nc.gpsimd.memset(negones_b[:], -1.0)
            nego2 = cpool.tile([P, 2, P], fp8)
            nc.gpsimd.memset(nego2[:], -1.0)
            ones1_b = cpool.tile([1, P], bf16)
            nc.gpsimd.memset(ones1_b[:], 1.0)
            warm = cpool.tile([1, 1], f32)
            # touch ACT early so its function-table DMA (~2.7us) is off the
            # critical path by the time the first pass1 runs
            nc.scalar.activation(warm[:], negones_f[0:1, 0:1], Identity,
                                 bias=0.0, scale=1.0)

            # ---- PE warmup: dummy matmuls so HAM un-throttles by the time
            # real work arrives (fits inside the chunk-0 DMA wait) ----
            warm_b = cpool.tile([P, 512], bf16)
            nc.gpsimd.memset(warm_b[:], 0.0)
            # tiny 1-col matmuls wake the PE clock at ~1/4 the PE-busy cost
            # of full 512-col dummies (PE duty is HAM-limited)
            warm_ps = psmall.tile([P, 512], f32, tag="w2ps", bufs=1)
            for _ in range(6):
                nc.tensor.matmul(warm_ps[:], warm_b[:, 0:P], warm_b[:],
                                 start=True, stop=True)

            # xbf blocked [p, blk, k, 256] so each block DMA lands in a
            # contiguous 2KB-per-partition stripe
            xbf = xpool.tile([P, NBLK, KT, BLK], fp8)
            wbf = wpool.tile([P, KT, C], fp8)
            xf16s = {}

            def dma_x16(hb):
                xf = x16pool.tile([P, KT, BLK], f16, tag="xf",
                                  name=f"xf{hb}", bufs=6)
                nc.sync.dma_start(xf[:], xT16r[hb])
                xf16s[hb] = xf

            def dma_xbf(blk):
                nc.sync.dma_start(xbf[:, blk, :, :], xT8r[blk])

            def dma_w(g):
                nc.sync.dma_start(wbf[:, 2 * g:2 * g + 2, :],
                                  wTr[:, 2 * g:2 * g + 2, :])

            # ring order: chunk-0 x2-chain data first (the square->tree->dot
            # chain needs a head start or pass1(b0) arrives after the PSUM
            # runway is exhausted), then first-matmul operands; everything
            # 2D-contiguous per partition
            dma_x16(0)
            dma_w(0)
            dma_xbf(0)
            dma_w(1)
            dma_xbf(1)
            dma_w(2)
            dma_w(3)
            dma_x16(1)
            dma_x16(2)
            dma_x16(3)
            dma_xbf(2)
            dma_xbf(3)
            dma_x16(4)
            dma_x16(5)
            for blk in range(4, NBLK):
                dma_xbf(blk)
            dma_x16(6)
            dma_x16(7)

            # ---- per-b-tile x^2 partials: GPSIMD squares + DVE k-trees;
            # t1 in bf16 so the x2-dot LDWEIGHTS runs at 1 cyc/row ----
            t1s = {}

            def prep_sq(ch):
                for jj in range(JT):
                    j = ch * JT + jj
                    xf = xf16s[j // 2] if jj % 2 == 0 else xf16s.pop(j // 2)
                    sl = slice((jj % 2) * P, (jj % 2) * P + P)
                    xsq = xsqpool.tile([P, KT, P], f32, tag="xsq",
                                       name=f"xsq{j}")
                    nc.gpsimd.tensor_tensor(xsq[:], xf[:, :, sl],
                                            xf[:, :, sl], op=MULT)
                    t4 = xsqpool.tile([P, 4, P], f32, tag="t4",
                                      name=f"t4_{j}")
                    nc.vector.tensor_tensor(t4[:], xsq[:, 0:4, :],
                                            xsq[:, 4:8, :], op=ADD)
                    t2 = xsqpool.tile([P, 2, P], f32, tag="t2",
                                      name=f"t2_{j}")
                    nc.vector.tensor_tensor(t2[:], t4[:, 0:2, :],
                                            t4[:, 2:4, :], op=ADD)
                    t1 = xsqpool.tile([P, P], bf16, tag="t1", bufs=8,
                                      name=f"t1_{j}")
                    nc.vector.tensor_tensor(t1[:], t2[:, 0, :], t2[:, 1, :],
                                            op=ADD)
                    t1s[j] = t1
                    yield j

            # ---- W^2 squares: e4m3 in/out, k0..k3 on DVE (interleaved with
            # chunk-0 trees), k4..k7 on ACT (after odd-tile pass1s) ----
            wsq = wpool.tile([P, KT, C], fp8)

            def wsq_act(k):
                nc.scalar.activation(wsq[:, k, :], wbf[:, k, :], Square)

            def wsq_dve(k):
                nc.vector.tensor_tensor(wsq[:, k, :], wbf[:, k, :],
                                        wbf[:, k, :], op=MULT)

            # k0..k3 on DVE interleaved with the chunk-0 trees; k4..k7 on
            # ACT after odd-tile pass1s. (Placing k4/k5 in ACT's pre-pass1
            # idle window measured worse: when the ring runs slow the W
            # groups land after the window closes and the stalled squares
            # block pass1(b0) -> PE PSUM starvation. In-loop is robust.
            # Deferring k2/k3 behind the chunk-1 trees measured ambiguous-
            # to-worse under thermal drift; this arrangement is the one
            # directly verified fastest.)
            for i, _ in enumerate(prep_sq(0)):
                wsq_dve(i)
            for _ in prep_sq(1):
                pass

            # ---- per-b-tile pieces ----
            y_bufs = {}
            t_bufs = {}

            def btile_matmuls(jg):
                """The 16 accumulating DR matmuls for one 128-row b-tile.

                Each cj gets its own single-bank [P,512] PSUM tile so ACT
                pass1 frees banks one ~0.6us op at a time instead of after
                a full 1024-col drain -- in steady state all four banks for
                the next tile are free before its first matmul needs them.
                """
                pss = [pmain.tile([P, 512], f32, tag="ps",
                                  name=f"ps{jg}_{c}") for c in range(4)]
                blk, sub = jg // 2, jg % 2
                # cj-OUTER: bank cj closes at the (cj+1)/4 mark of the group
                # instead of all four closing at the end, so ACT pass1 (bias
                # pre-hoisted) drains banks DURING the group and the next
                # tile finds its banks recycled before it needs them
                for cj in range(4):
                    for k2 in range(KT // 2):
                        lhsT = xbf[:, blk, 2 * k2:2 * k2 + 2,
                                   sub * P:(sub + 1) * P]
                        nc.tensor.matmul(
                            pss[cj][:],
                            lhsT,
                            wbf[:, 2 * k2:2 * k2 + 2, cj * 512:(cj + 1) * 512],
                            start=(k2 == 0),
                            stop=(k2 == KT // 2 - 1),
                            perf_mode=DR,
                        )
                y_bufs[jg] = pss

            def x2_col(jg):
                """x2 column (-sum(x^2)/D) for one b-tile: PE dot + ACT copy."""
                t1 = t1s.pop(jg)
                x2ps = psmall.tile([P, 1], f32, tag="x2ps", bufs=1,
                                   name=f"x2ps{jg}")
                nc.tensor.matmul(x2ps[:], t1[:], negones_b[:],
                                 start=True, stop=True)
                x2c = spool.tile([P, 1], f32, tag="x2c", name=f"x2c{jg}")
                nc.scalar.activation(x2c[:], x2ps[:], Copy, bias=0.0,
                                     scale=1.0 / D)
                return x2c

            def btile_pass1(jg, x2c):
                """ACT: t = cross_scale*psum - x2[b]  (f16 out, drains PSUM
                one 512-col bank per op)."""
                pss = y_bufs.pop(jg)
                ts = []
                for h in range(2):
                    t = epool.tile([P, 1024], f16, tag="t", name=f"t{jg}_{h}")
                    for q in range(2):
                        nc.scalar.activation(t[:, q * 512:(q + 1) * 512],
                                             pss[2 * h + q][:], Identity,
                                             bias=x2c[:], scale=cross_scale)
                    ts.append(t)
                t_bufs[jg] = ts

            def btile_pass2(jg, split=False):
                """DVE: y = t - w2rep (all f16) + store.

                split: store each 1024-col half as soon as it's ready so the
                last b-tile's final DMA overlaps the second half's epilogue.
                """
                ts = t_bufs.pop(jg)
                y_t = ypool.tile([P, C], f16, tag="y_t", name=f"y_t{jg}")
                for h in range(2):
                    ysl = y_t[:, h * 1024:(h + 1) * 1024]
                    nc.vector.tensor_add(
                        ysl, ts[h][:], w2rep[:, h * 1024:(h + 1) * 1024]
                    )
                    if split:
                        nc.sync.dma_start(
                            y[jg * P:(jg + 1) * P, h * 1024:(h + 1) * 1024],
                            ysl,
                        )
                if not split:
                    nc.sync.dma_start(y[jg * P:(jg + 1) * P, :], y_t[:])

            def btile_tail(jg, x2c):
                """Last b-tile: per-half pass1 -> pass2 -> store pipeline so
                ACT, DVE and the ring overlap maximally in the tail."""
                pss = y_bufs.pop(jg)
                y_t = ypool.tile([P, C], f16, tag="y_t", name=f"y_t{jg}")
                for h in range(2):
                    t = epool.tile([P, 1024], f16, tag="t", name=f"t{jg}_{h}")
                    for q in range(2):
                        nc.scalar.activation(t[:, q * 512:(q + 1) * 512],
                                             pss[2 * h + q][:], Identity,
                                             bias=x2c[:], scale=cross_scale)
                    ysl = y_t[:, h * 1024:(h + 1) * 1024]
                    nc.vector.tensor_add(
                        ysl, t[:], w2rep[:, h * 1024:(h + 1) * 1024]
                    )
                    nc.sync.dma_start(
                        y[jg * P:(jg + 1) * P, h * 1024:(h + 1) * 1024],
                        ysl,
                    )

            def w2_finish():
                """DR reduce of wsq + broadcast: w2rep [P, C] f16."""
                w2row = wpool.tile([1, C], bf16)
                for cj in range(4):
                    w2ps = psmall.tile([P, 512], f32, tag="w2ps", bufs=1,
                                       name=f"w2ps{cj}")
                    for k2 in range(KT // 2):
                        nc.tensor.matmul(
                            w2ps[:],
                            nego2[:],
                            wsq[:, 2 * k2:2 * k2 + 2,
                                cj * 512:(cj + 1) * 512],
                            start=(k2 == 0),
                            stop=(k2 == KT // 2 - 1),
                            perf_mode=DR,
                        )
                    # w2row = -sum(W^2)/D (every PSUM row holds the sum)
                    nc.scalar.activation(w2row[:, cj * 512:(cj + 1) * 512],
                                         w2ps[0:1, :], Copy, bias=0.0,
                                         scale=w2_scale)
                rep = wpool.tile([P, C], f16)
                for cj in range(4):
                    w2rp = psmall.tile([P, 512], f32, tag="w2ps", bufs=1,
                                       name=f"w2rp{cj}")
                    nc.tensor.matmul(w2rp[:], ones1_b[:],
                                     w2row[:, cj * 512:(cj + 1) * 512],
                                     start=True, stop=True)
                    nc.scalar.activation(rep[:, cj * 512:(cj + 1) * 512],
                                         w2rp[:], Copy, bias=0.0, scale=1.0)
                return rep

            # ---- main per-b-tile loop ----
            w2rep = None
            for j in range(NJ):
                if j > 0:
                    # bias pre-hoisted (t1 is ready a chunk early) so pass1
                    # can drain banks as each cj accumulation closes
                    x2c = x2_col(j)
                btile_matmuls(j)
                if j == 0:
                    x2c = x2_col(j)
                if j == NJ - 1:
                    btile_tail(j, x2c)
                    continue
                btile_pass1(j, x2c)
                if j in (1, 3, 5, 7):
                    # k4..k7 on odd tiles so ACT's per-tile load stays ~=
                    # the PE's b-tile cadence (pass1 is what frees PSUM);
                    # half-slice grain spread over j=1..7 measured worse
                    # (insertions land on the fragile b1/b2 tiles)
                    wsq_act(4 + (j - 1) // 2)
                if j == 7:
                    w2rep = w2_finish()
                    for _ in prep_sq(2):
                        pass
                    for _ in prep_sq(3):
                        pass
                    for jj in range(8):     # deferred chunk-0/1 stores
                        btile_pass2(jj)
                if j >= 8:
                    btile_pass2(j, split=(j == NJ - 2))

    nc.compile()
    return nc


def _get_nc():
    if "nc" not in _CACHE:
        _CACHE["nc"] = _build_nc()
    return _CACHE["nc"]


def _prep_inputs(x, W):
    x = np.ascontiguousarray(x, dtype=np.float32)
    W = np.ascontiguousarray(W, dtype=np.float32)
    # W -> [p, k, c] p-major, x16 prescale, e4m3
    wp = W.reshape(C, KT, P).transpose(2, 1, 0)  # [p, k, c]
    wT = (np.ascontiguousarray(wp) * np.float32(16.0)).astype(
        ml_dtypes.float8_e4m3).reshape(P, KT * C)
    in_maps = []
    for i in range(NCORES):
        xs = x[i * BSH:(i + 1) * BSH, :]             # [BSH, D]
        # xT8: b-256-blocked [blk, p, k, b], x16 prescale
        x8 = xs.reshape(NBLK, BLK, KT, P).transpose(0, 3, 2, 1)
        xT8_i = (np.ascontiguousarray(x8) * np.float32(16.0)).astype(
            ml_dtypes.float8_e4m3).reshape(NBLK * P, KT * BLK)
        # xT16: b-256-blocked [hb, p, k, b] (2 b-tiles per block)
        x16 = xs.reshape(NBLK, BLK, KT, P).transpose(0, 3, 2, 1)
        xT16_i = np.ascontiguousarray(x16).astype(np.float16).reshape(
            NBLK * P, KT * BLK)
        in_maps.append({"xT8": xT8_i, "xT16": xT16_i, "wT": wT})
    return in_maps


def run(x, W, trace=False, **trace_kwargs):
    """Run on the 8 cores; returns (out [B, C] f32, BassKernelResults)."""
    from concourse import bass_utils

    nc = _get_nc()
    in_maps = _prep_inputs(x, W)
    res = bass_utils.run_bass_kernel_spmd(
        nc, in_maps, core_ids=list(range(NCORES)), trace=trace, **trace_kwargs
    )
    out = np.concatenate(
        [r["y"].astype(np.float32) for r in res.results], axis=0
    )
    return out, res


def kernel(x, W, task_id=None, **_unused):
    out, _ = run(np.asarray(x), np.asarray(W), trace=False)
    return out
